# revision 9
# baseline (speedup 1.0000x reference)
"""Trainium2 Bass kernel for nn_DQNNetwork (gnn_message_passing).

Reference computation (fp32):
    h  = relu(x @ Wh.T + bh)                       # [n, 512]
    mo = (sum_j h[j] - h) / (n - 1)                # leave-one-out mean
    out = relu(concat([h, mo], 1) @ Wf.T + bf)     # [n, 3] -> flat

Algebraic restructuring (exact up to fp rounding): with Wf = [Wf1 | Wf2],
S = colsum(h), W' = Wf1 - Wf2/(n-1), c = S @ (Wf2.T/(n-1)) + bf:
    out = relu(h @ W'.T + c)
so the only cross-device coupling is c (3 floats) -> one tiny AllGather.

Sharding: data-parallel over rows. 8 cores x 8192 rows. Weights replicated.

v2 layout: the host pre-transposes and packs x into hT-feed order
[128, (block, kchunk, row)] fp16, so the kernel does ZERO on-chip
transposes (saves ~74k PE cycles = 24% of PE time vs v1) and half the
HBM traffic. Per-core dataflow:
  phase A (16 blocks of 512 rows): DMA xT block (one contiguous 6KB/
    partition descriptor) -> fp16 GEMM1 (WhT stationary) -> hT in PSUM
    -> ACT relu+bias, fp32 colsum accum -> hT fp16 kept in SBUF (64KB/p).
  then: colsum reduce -> c_loc = S_loc @ Wf2s.T (tiny PE matvec) ->
    AllGather of 3 floats -> c, all overlapped under phase B's GEMM2.
  phase B (16 blocks): GEMM2 (W'T stationary, hT moving) -> [3, 512]
    PSUM. Early blocks (< CUT, before c lands) buffer pre-activations
    and relu+c on DVE once c arrives; late blocks relu+c directly from
    PSUM on ACT. Output DMA streams per block -> no serial tail.

`rep` repeats the whole per-core pipeline (weights loaded once) so
wall-clock deltas between rep values isolate kernel time from the axon
RPC overhead.
"""

import numpy as np

import concourse.bacc as bacc
import concourse.mybir as mybir
import concourse.tile as tile
from concourse import bass_utils

N_CORES = 8
N = 65536               # total rows (stocks)
F = 768                 # input features
H = 512                 # hidden features
A = 3                   # actions
R = N // N_CORES        # rows per core = 8192
RB = 512                # rows per block
NB = R // RB            # blocks per core = 16
KF = F // 128           # feature chunks = 6
KH = H // 128           # hidden chunks = 4
CUT = 8                 # blocks whose final relu waits for c on DVE

F32 = mybir.dt.float32
F16 = mybir.dt.float16
RELU = mybir.ActivationFunctionType.Relu

_cache = {}


def build_module(rep=1, collective=True, num_devices=N_CORES):
    key = (rep, collective, num_devices)
    if key in _cache:
        return _cache[key]

    nc = bacc.Bacc("TRN2", target_bir_lowering=False, debug=False,
                   num_devices=num_devices)

    x = nc.dram_tensor("x", [128, NB * KF * RB], F16,
                       kind="ExternalInput").ap()
    wht = nc.dram_tensor("wht", [F, H], F16, kind="ExternalInput").ap()
    bh_t = nc.dram_tensor("bh_t", [128, KH], F32, kind="ExternalInput").ap()
    wpt = nc.dram_tensor("wpt", [128, KH * A], F16, kind="ExternalInput").ap()
    wf2t = nc.dram_tensor("wf2t", [128, KH * A], F32,
                          kind="ExternalInput").ap()
    bf = nc.dram_tensor("bf", [A, 1], F32, kind="ExternalInput").ap()
    y = nc.dram_tensor("out", [A, R], F32, kind="ExternalOutput").ap()

    with tile.TileContext(nc) as tc:
        with (
            tc.tile_pool(name="const", bufs=1) as const,
            tc.tile_pool(name="xin", bufs=3) as xin_pool,
            tc.tile_pool(name="ph", bufs=3, space="PSUM") as ph_pool,
            tc.tile_pool(name="p2", bufs=4, space="PSUM") as p2_pool,
            tc.tile_pool(name="dram", bufs=1, space="DRAM") as dram,
        ):
            wht_sb = const.tile([128, KF * H], F16)
            wht_r = wht.rearrange("(k p) h -> p k h", p=128)
            for k in range(KF):  # per-chunk so cold-start GEMM1 begins early
                nc.scalar.dma_start(out=wht_sb[:, k * H:(k + 1) * H],
                                    in_=wht_r[:, k])
            bh_sb = const.tile([128, KH], F32)
            nc.scalar.dma_start(out=bh_sb[:], in_=bh_t[:])
            wpt_sb = const.tile([128, KH * A], F16)
            nc.scalar.dma_start(out=wpt_sb[:], in_=wpt[:])
            wf2t_sb = const.tile([128, KH * A], F32)
            nc.scalar.dma_start(out=wf2t_sb[:], in_=wf2t[:])
            bf_sb = const.tile([A, 1], F32)
            nc.scalar.dma_start(out=bf_sb[:], in_=bf[:])

            ht_all = const.tile([128, KH * R], F16)   # hT, whole shard
            s_parts = const.tile([128, KH * NB], F32)  # colsum per (m, b)
            s_loc = const.tile([128, KH], F32)
            pre2 = const.tile([A, CUT * RB], F32)
            out_sb = const.tile([A, R], F32)
            c_loc = const.tile([A, 1], F32)
            c_all = const.tile([A, num_devices], F32)
            c_red = const.tile([A, 1], F32)
            c_sb = const.tile([A, 1], F32)

            for _rep in range(rep):
                # ---- phase A: GEMM1 + relu(+bias) + colsum accumulation
                for b in range(NB):
                    x_sb = xin_pool.tile([128, KF * RB], F16)
                    if _rep == 0 and b == 0:
                        # split the cold-start load so GEMM1 starts after
                        # the first k-chunk instead of the whole block
                        for k in range(KF):
                            nc.sync.dma_start(
                                out=x_sb[:, k * RB:(k + 1) * RB],
                                in_=x[:, k * RB:(k + 1) * RB])
                    else:
                        nc.sync.dma_start(
                            out=x_sb[:],
                            in_=x[:, b * KF * RB:(b + 1) * KF * RB])
                    # m-outer / k-inner: each m-chunk finishes a full
                    # accumulation pass before the next starts, so ACT has
                    # a whole pass (~1.3us) to drain each PSUM bank and the
                    # 3-deep ring never back-pressures PE.
                    for m in range(KH):
                        ph = ph_pool.tile([128, RB], F32, tag="ph",
                                          name=f"ph{m}_{b}")
                        for k in range(KF):
                            nc.tensor.matmul(
                                ph[:],
                                wht_sb[:, k * H + m * 128:
                                       k * H + (m + 1) * 128],
                                x_sb[:, k * RB:(k + 1) * RB],
                                start=(k == 0), stop=(k == KF - 1))
                        nc.scalar.activation(
                            ht_all[:, m * R + b * RB:m * R + (b + 1) * RB],
                            ph[:], RELU, bias=bh_sb[:, m:m + 1],
                            accum_out=s_parts[:, m * NB + b:m * NB + b + 1])

                # ---- local colsum (DVE) overlaps first GEMM2 blocks on PE
                nc.vector.tensor_reduce(
                    s_loc[:], s_parts[:].rearrange("p (m b) -> p m b", b=NB),
                    axis=mybir.AxisListType.X, op=mybir.AluOpType.add)

                def gemm2(b):
                    p2 = p2_pool.tile([A, RB], F32, name=f"p2_{b}", tag="p2")
                    for m in range(KH):
                        nc.tensor.matmul(
                            p2[:], wpt_sb[:, m * A:(m + 1) * A],
                            ht_all[:, m * R + b * RB:m * R + (b + 1) * RB],
                            start=(m == 0), stop=(m == KH - 1))
                    return p2

                early = [gemm2(b) for b in range(2)]

                # c_loc = S_loc @ (Wf2.T/(n-1)): tiny PE matvec, then the
                # 12-byte AllGather; lands while PE grinds through GEMM2.
                pc = p2_pool.tile([A, RB], F32, tag="p2", name="pc")
                for m in range(KH):
                    nc.tensor.matmul(pc[:, 0:1], wf2t_sb[:, m * A:(m + 1) * A],
                                     s_loc[:, m:m + 1],
                                     start=(m == 0), stop=(m == KH - 1))
                nc.vector.tensor_copy(c_loc[:], pc[:, 0:1])
                if collective:
                    ar_in = dram.tile([A, 1], F32, name=f"ar_in_{_rep}",
                                      tag=f"ar_in_{_rep}")
                    ag_out = dram.tile([num_devices * A, 1], F32,
                                       addr_space="Shared",
                                       name=f"ag_out_{_rep}",
                                       tag=f"ag_out_{_rep}")
                    nc.sync.dma_start(out=ar_in[:], in_=c_loc[:])
                    nc.gpsimd.collective_compute(
                        "AllGather", mybir.AluOpType.bypass,
                        replica_groups=[list(range(num_devices))],
                        ins=[ar_in.opt()], outs=[ag_out.opt()],
                    )
                    nc.sync.dma_start(
                        out=c_all[:],
                        in_=ag_out[:].rearrange("(r a) one -> a (r one)",
                                                a=A))

                # ---- phase B: GEMM2 + relu(+c) + streamed output DMA
                for b, p2 in enumerate(early):
                    nc.vector.tensor_copy(pre2[:, b * RB:(b + 1) * RB],
                                          p2[:])
                for b in range(2, NB):
                    p2 = gemm2(b)
                    if b < CUT:
                        nc.vector.tensor_copy(pre2[:, b * RB:(b + 1) * RB],
                                              p2[:])
                    else:
                        if b == CUT:
                            # c = sum_r c_loc_r + bf. Emitted here (not at
                            # the collective) so the in-order DVE queue
                            # drains the early-block PSUM copies without
                            # waiting on the AllGather.
                            if collective:
                                nc.vector.tensor_reduce(
                                    c_red[:], c_all[:],
                                    axis=mybir.AxisListType.X,
                                    op=mybir.AluOpType.add)
                                nc.vector.tensor_add(c_sb[:], c_red[:],
                                                     bf_sb[:])
                            else:
                                nc.vector.tensor_add(c_sb[:], c_loc[:],
                                                     bf_sb[:])
                        nc.scalar.activation(out_sb[:, b * RB:(b + 1) * RB],
                                             p2[:], RELU, bias=c_sb[:])
                        nc.scalar.dma_start(
                            out=y[:, b * RB:(b + 1) * RB],
                            in_=out_sb[:, b * RB:(b + 1) * RB])
                # early blocks: relu(pre2 + c) on DVE once c arrives
                for b in range(CUT):
                    nc.vector.tensor_scalar(
                        out_sb[:, b * RB:(b + 1) * RB],
                        pre2[:, b * RB:(b + 1) * RB],
                        scalar1=c_sb[:], scalar2=0.0,
                        op0=mybir.AluOpType.add, op1=mybir.AluOpType.max)
                    nc.sync.dma_start(out=y[:, b * RB:(b + 1) * RB],
                                      in_=out_sb[:, b * RB:(b + 1) * RB])

    nc.compile()
    _cache[key] = nc
    return nc


def prepare_in_maps(x, Wh, bh, Wf, bf):
    x = np.asarray(x, dtype=np.float32)
    Wh = np.asarray(Wh, dtype=np.float32)
    bh = np.asarray(bh, dtype=np.float32)
    Wf = np.asarray(Wf, dtype=np.float32)
    bf = np.asarray(bf, dtype=np.float32)

    inv = np.float32(1.0) / np.float32(N - 1)
    Wf1 = Wf[:, :H]
    Wf2s = Wf[:, H:] * inv                      # [3, 512] scaled
    Wp = Wf1 - Wf2s                             # [3, 512]

    def chunk_t(w, dt):                         # [A, 512] -> [128, KH*A]
        return np.ascontiguousarray(
            w.T.reshape(KH, 128, A).transpose(1, 0, 2).reshape(128, KH * A),
            dtype=dt)

    wht = np.ascontiguousarray(Wh.T, dtype=np.float16)       # [768, 512]
    bh_t = np.ascontiguousarray(bh.reshape(KH, 128).T)       # [128, 4]
    wpt = chunk_t(Wp, np.float16)
    wf2t = chunk_t(Wf2s, np.float32)
    bf_c = np.ascontiguousarray(bf.reshape(A, 1))

    shared = {"wht": wht, "bh_t": bh_t, "wpt": wpt, "wf2t": wf2t, "bf": bf_c}

    xh = x.astype(np.float16)
    in_maps = []
    for c in range(N_CORES):
        # pack shard transpose as [128, (block, kchunk, row)]
        xt = xh[c * R:(c + 1) * R].T                  # [768, 8192] view
        xp = np.ascontiguousarray(
            xt.reshape(KF, 128, NB, RB).transpose(1, 2, 0, 3)
              .reshape(128, NB * KF * RB))
        in_maps.append({"x": xp, **shared})
    return in_maps


def gather(results):
    full = np.empty((N, A), dtype=np.float32)
    for c, res in enumerate(results):
        full[c * R:(c + 1) * R, :] = res["out"].T
    return full.reshape(-1)


def kernel(x, Wh, bh, Wf, bf):
    nc = build_module()
    in_maps = prepare_in_maps(x, Wh, bh, Wf, bf)
    res = bass_utils.run_bass_kernel_spmd(nc, in_maps,
                                          core_ids=list(range(N_CORES)))
    return gather(res.results)
